# revision 3
# baseline (speedup 1.0000x reference)
"""Trainium2 Bass kernel for the hypernet-MoE model (nn_BaseModel_53455162966557).

Math (per sample b):
    h  = relu(relu(x @ W0 + b0) @ W1 + b1)                    [B, D]
    c  = relu(context @ Wh1 + bh1)                            [B, H]
    flat = c @ Wh2 + bh2                                      [B, NPARAMS]
    z  = relu(einsum(h, flat[:, :i0] as [D, M]) + flat[:, i0:i1])
    z2 = einsum(z, flat[:, i1:i2] as [M, D]) + flat[:, i2:]
    out = relu(h + z2)

v2 restructuring ("scaled-lhsT"): instead of computing A_k = h @ Wh2_k per
hypernet unit k and then scale-accumulating c[b,k]*A_k on DVE/ACT (the v1
bottleneck: ~500 ns x 1024 PSUM-offload ops), fold the per-sample gate into
the matmul LHS:

    z = sum_k (c[:,k] * h) @ Wh2_k   -- PSUM accumulates over all k directly.

The per-k scaled lhsT tiles bf16(c_k * h) are built by DVE tensor_mul
(~330 ns, all-SBUF 2x mode) against a gate table cbT[128, k, b] that holds
c^T replicated across partitions.  cbT is built once by GPSIMD
partition_broadcast (attn library) from a single-partition copy of c^T
(round-tripped through a DRAM scratch tensor), and is reused by both layers.
Wh2 streams from HBM in fp8 e4m3 (x256 host scale), halving DMA bytes; the
stream matmuls are bf16 lhsT x fp8 rhs (full-rate, validated on HW).

Sharding: pure data parallel, batch 2048 -> 8 cores x 256.
"""

import ml_dtypes
import numpy as np

import concourse.bass as bass
import concourse.tile as tile
from concourse import bacc, library_config, mybir
from concourse.masks import make_identity

F32 = mybir.dt.float32
BF16 = mybir.dt.bfloat16
F8 = mybir.dt.float8e4
AF = mybir.ActivationFunctionType
ALU = mybir.AluOpType

B, OBS, CTX, D, M, H = 2048, 64, 16, 256, 256, 256
NCORES = 8
BS = B // NCORES  # 256 rows per core
I0 = D * M
I1 = I0 + M
I2 = I1 + M * D
NPARAMS = I2 + D
G = 8  # k values per stream DMA group
NG = H // G  # 32
NB = 16  # k values per partition_broadcast op
SW = 256.0  # host-side fp8 scale on streamed Wh2 blocks
# cflat j-th 256-block holds cT partition p=j//2, chunk kc=j%2 -> true k:
KPERM = [(j % 2) * 128 + j // 2 for j in range(H)]

_CACHED_NC = None


def build_nc():
    nc = bacc.Bacc("TRN2", target_bir_lowering=False, debug=False)

    x = nc.dram_tensor("x", [BS, OBS], F32, kind="ExternalInput")
    ctx_in = nc.dram_tensor("context", [BS, CTX], F32, kind="ExternalInput")
    W0 = nc.dram_tensor("W0", [OBS, D], F32, kind="ExternalInput")
    b0 = nc.dram_tensor("b0", [D], F32, kind="ExternalInput")
    W1 = nc.dram_tensor("W1", [D, D], F32, kind="ExternalInput")
    b1 = nc.dram_tensor("b1", [D], F32, kind="ExternalInput")
    Wh1 = nc.dram_tensor("Wh1", [CTX, H], F32, kind="ExternalInput")
    bh1 = nc.dram_tensor("bh1", [H], F32, kind="ExternalInput")
    Wh2s8 = nc.dram_tensor("Wh2s8", [2, NG, 128 * G * 2 * 256], F8,
                           kind="ExternalInput")
    Wh2e = nc.dram_tensor("Wh2e", [H, 512], BF16, kind="ExternalInput")
    BW1 = nc.dram_tensor("BW1", [128, 2, M], BF16, kind="ExternalInput")
    BW2 = nc.dram_tensor("BW2", [128, 2, D], BF16, kind="ExternalInput")
    bb1d = nc.dram_tensor("bb1", [1, M], BF16, kind="ExternalInput")
    bb2d = nc.dram_tensor("bb2", [1, D], BF16, kind="ExternalInput")
    cscr = nc.dram_tensor("cscr", [128, 2, 256], BF16, kind="Internal")
    out = nc.dram_tensor("out", [BS, D], F32, kind="ExternalOutput")

    with tile.TileContext(nc) as tc:
        nc.gpsimd.load_library(library_config.attn)
        with (
            tc.tile_pool(name="consts", bufs=1) as consts,
            tc.tile_pool(name="wts", bufs=1) as wts,
            tc.tile_pool(name="acts", bufs=1) as acts,
            tc.tile_pool(name="cbt", bufs=1) as cbtp,
            tc.tile_pool(name="cfl", bufs=2) as cflp,
            tc.tile_pool(name="sc", bufs=8) as scp,
            tc.tile_pool(name="wh2s", bufs=4) as wh2s,
            tc.tile_pool(name="pz", bufs=2, space="PSUM") as pzp,
            tc.tile_pool(name="pmisc", bufs=2, space="PSUM") as pmisc,
        ):
            # ---- input activations first: they head the critical path ----
            xsb = acts.tile([128, 2, OBS], F32)
            nc.sync.dma_start(out=xsb[:], in_=x[:, :].rearrange("(hb p) o -> p hb o", p=128))
            ctxsb = acts.tile([128, 2, CTX], F32)
            nc.sync.dma_start(out=ctxsb[:], in_=ctx_in[:, :].rearrange("(hb p) o -> p hb o", p=128))
            W0sb = wts.tile([OBS, D], F32)
            nc.sync.dma_start(out=W0sb[:], in_=W0[:])
            W1sb = wts.tile([128, 2, D], F32)
            nc.sync.dma_start(out=W1sb[:], in_=W1[:, :].rearrange("(cc p) j -> p cc j", p=128))
            Wh1sb = wts.tile([CTX, H], F32)
            nc.sync.dma_start(out=Wh1sb[:], in_=Wh1[:])
            b0sb = wts.tile([128, 2], F32)
            nc.sync.dma_start(out=b0sb[:], in_=b0[:].rearrange("(cc p) -> p cc", p=128))
            b1sb = wts.tile([128, 2], F32)
            nc.sync.dma_start(out=b1sb[:], in_=b1[:].rearrange("(cc p) -> p cc", p=128))
            bh1sb = wts.tile([128, 2], F32)
            nc.sync.dma_start(out=bh1sb[:], in_=bh1[:].rearrange("(cc p) -> p cc", p=128))

            # init-matmul weights (bf16, host-scaled by SW)
            Wh2esb = wts.tile([128, 2, 512], BF16)
            nc.sync.dma_start(
                out=Wh2esb[:], in_=Wh2e[:, :].rearrange("(cc p) f -> p cc f", p=128)
            )
            B1sb = wts.tile([128, 2, M], BF16)
            nc.sync.dma_start(out=B1sb[:], in_=BW1[:])
            B2sb = wts.tile([128, 2, D], BF16)
            nc.sync.dma_start(out=B2sb[:], in_=BW2[:])
            bb1 = wts.tile([1, M], BF16)
            nc.sync.dma_start(out=bb1[:], in_=bb1d[:])
            bb2 = wts.tile([1, D], BF16)
            nc.sync.dma_start(out=bb2[:], in_=bb2d[:])

            ident = consts.tile([128, 128], F32)
            make_identity(nc, ident[:])
            onesb = consts.tile([1, 128], BF16)
            nc.vector.memset(onesb[:], 1.0)

            # ---- transposes of x / context ----
            xT = acts.tile([OBS, BS], F32)
            ctxT = acts.tile([CTX, BS], F32)
            for hb in range(2):
                pt = pmisc.tile([128, 256], F32, tag="pm")
                nc.tensor.transpose(pt[0:OBS, 0:128], xsb[:, hb, :], ident[:])
                nc.vector.tensor_copy(xT[:, hb * 128:(hb + 1) * 128], pt[0:OBS, 0:128])
                pt2 = pmisc.tile([128, 256], F32, tag="pm")
                nc.tensor.transpose(pt2[0:CTX, 0:128], ctxsb[:, hb, :], ident[:])
                nc.vector.tensor_copy(ctxT[:, hb * 128:(hb + 1) * 128], pt2[0:CTX, 0:128])

            # ---- main MLP: hT = relu(W1.T @ relu(W0.T @ xT + b0) + b1) ----
            h1T = acts.tile([128, 2, BS], F32)
            for dc in range(2):
                ph = pmisc.tile([128, 256], F32, tag="pm")
                nc.tensor.matmul(
                    ph[:], W0sb[:, dc * 128:(dc + 1) * 128], xT[:], start=True, stop=True
                )
                nc.scalar.activation(h1T[:, dc, :], ph[:], AF.Relu, bias=b0sb[:, dc:dc + 1])
            hT = acts.tile([128, 2, BS], F32)
            for dc2 in range(2):
                ph = pmisc.tile([128, 256], F32, tag="pm")
                nc.tensor.matmul(
                    ph[:], W1sb[:, 0, dc2 * 128:(dc2 + 1) * 128], h1T[:, 0, :],
                    start=True, stop=False,
                )
                nc.tensor.matmul(
                    ph[:], W1sb[:, 1, dc2 * 128:(dc2 + 1) * 128], h1T[:, 1, :],
                    start=False, stop=True,
                )
                nc.scalar.activation(hT[:, dc2, :], ph[:], AF.Relu, bias=b1sb[:, dc2:dc2 + 1])
            hTb = acts.tile([128, 2, BS], BF16)
            nc.vector.tensor_copy(hTb[:], hT[:])

            # ---- hypernet first layer: cT = relu(Wh1.T @ ctxT + bh1) ----
            cT = acts.tile([128, 2, BS], F32)
            for cc in range(2):
                ph = pmisc.tile([128, 256], F32, tag="pm")
                nc.tensor.matmul(
                    ph[:], Wh1sb[:, cc * 128:(cc + 1) * 128], ctxT[:], start=True, stop=True
                )
                nc.scalar.activation(cT[:, cc, :], ph[:], AF.Relu, bias=bh1sb[:, cc:cc + 1])
            cTb = acts.tile([128, 2, BS], BF16)
            nc.vector.tensor_copy(cTb[:], cT[:])

            # ---- h (b-major, for the skip connection) ----
            hsb = acts.tile([128, 2, D], F32)
            for dc in range(2):
                for hb in range(2):
                    pt = pmisc.tile([128, 256], F32, tag="pm")
                    nc.tensor.transpose(
                        pt[:, 0:128], hT[:, dc, hb * 128:(hb + 1) * 128], ident[:]
                    )
                    nc.vector.tensor_copy(hsb[:, hb, dc * 128:(dc + 1) * 128], pt[:, 0:128])

            # ---- gate table cbT[p, j, b] = cT[KPERM[j], b] for all p ----
            # cTb round-trips through DRAM to land on a single partition,
            # then GPSIMD broadcasts NB k-rows per op.
            nc.sync.dma_start(out=cscr[:], in_=cTb[:])
            cbT = cbtp.tile([128, H, 256], BF16)
            for gb in range(H // NB):
                cfl = cflp.tile([1, NB * 256], BF16, tag="cfl")
                nc.sync.dma_start(
                    out=cfl[:],
                    in_=cscr[gb * NB // 2:(gb + 1) * NB // 2, :, :]
                    .rearrange("p kc b -> () (p kc b)"),
                )
                nc.gpsimd.partition_broadcast(
                    cbT[:, gb * NB:(gb + 1) * NB, :], cfl[0:1, :]
                )

            def stream_layer(layer, lhsTb, zps):
                """z[hb] = sum_k (c_k*lhs) @ Wh2_k + bias terms, in PSUM."""
                ecol = 0 if layer == 0 else 256
                Bsb = B1sb if layer == 0 else B2sb
                bbs = bb1 if layer == 0 else bb2
                for hb in range(2):
                    sl = slice(hb * 128, (hb + 1) * 128)
                    zp = zps[hb][:]
                    nc.tensor.matmul(zp, cTb[:, 0, sl], Wh2esb[:, 0, ecol:ecol + 256],
                                     start=True, stop=False)
                    nc.tensor.matmul(zp, cTb[:, 1, sl], Wh2esb[:, 1, ecol:ecol + 256],
                                     start=False, stop=False)
                    nc.tensor.matmul(zp, lhsTb[:, 0, sl], Bsb[:, 0, :],
                                     start=False, stop=False)
                    nc.tensor.matmul(zp, lhsTb[:, 1, sl], Bsb[:, 1, :],
                                     start=False, stop=False)
                    nc.tensor.matmul(zp, onesb[:], bbs[:], start=False, stop=False)
                for g in range(NG):
                    wt8 = wh2s.tile([128, 2, G, 256], F8, tag="wt8")
                    nc.sync.dma_start(
                        out=wt8[:],
                        in_=Wh2s8[layer, g, :].rearrange("(p f) -> p f", p=128),
                    )
                    for kin in range(G):
                        j = g * G + kin
                        scl = scp.tile([128, 2, 256], BF16, tag="scl")
                        nc.vector.tensor_mul(
                            scl[:], lhsTb[:],
                            cbT[:, j, :].unsqueeze(1).broadcast_to((128, 2, 256)),
                        )
                        last = j == H - 1
                        for hb in range(2):
                            for ch in range(2):
                                nc.tensor.matmul(
                                    zps[hb][:],
                                    scl[:, ch, hb * 128:(hb + 1) * 128],
                                    wt8[:, ch, kin, :],
                                    start=False, stop=(last and ch == 1),
                                )

            # ---- layer 1 ----
            zps1 = [pzp.tile([128, 256], F32, tag=f"z{hb}", name=f"zps1_{hb}") for hb in range(2)]
            stream_layer(0, hTb, zps1)
            zrel = acts.tile([128, 2, M], F32)
            for hb in range(2):
                nc.scalar.activation(zrel[:, hb, :], zps1[hb][:], AF.Relu, scale=1.0 / SW)
            zTb = acts.tile([128, 2, BS], BF16)
            for mc in range(2):
                for hb in range(2):
                    pt = pmisc.tile([128, 256], F32, tag="pm")
                    nc.tensor.transpose(
                        pt[:, 0:128], zrel[:, hb, mc * 128:(mc + 1) * 128], ident[:]
                    )
                    nc.vector.tensor_copy(zTb[:, mc, hb * 128:(hb + 1) * 128], pt[:, 0:128])

            # ---- layer 2 ----
            zps2 = [pzp.tile([128, 256], F32, tag=f"z{hb}", name=f"zps2_{hb}") for hb in range(2)]
            stream_layer(1, zTb, zps2)
            qh = acts.tile([128, 2, D], F32)
            orel = acts.tile([128, 2, D], F32)
            for hb in range(2):
                nc.vector.scalar_tensor_tensor(
                    qh[:, hb, :], zps2[hb][:], 1.0 / SW, hsb[:, hb, :],
                    op0=ALU.mult, op1=ALU.add,
                )
                nc.scalar.activation(orel[:, hb, :], qh[:, hb, :], AF.Relu)
            nc.sync.dma_start(
                out=out[:, :].rearrange("(hb p) d -> p hb d", p=128), in_=orel[:]
            )

    nc.compile()
    return nc


def _stage(Wh2, bh2):
    """Host-side staging: fp8 stream blocks (k-permuted to match the cflat
    partition-major order), bf16 bias/init operands, all pre-scaled by SW."""
    W1p = (Wh2[:, :I0] * SW).reshape(H, 2, 128, 256)   # k, dc, p, m
    W2p = (Wh2[:, I1:I2] * SW).reshape(H, 2, 128, 256)  # k, mc, p, d
    s = np.stack([W1p, W2p])[:, KPERM]                  # l, j, ch, p, cols
    s = s.transpose(0, 1, 3, 2, 4)                      # l, j, p, ch, cols
    s = s.reshape(2, NG, G, 128, 2, 256).transpose(0, 1, 3, 4, 2, 5)
    Wh2s8 = np.ascontiguousarray(s).astype(ml_dtypes.float8_e4m3)
    Wh2s8 = Wh2s8.reshape(2, NG, 128 * G * 2 * 256)
    bf = ml_dtypes.bfloat16
    Wh2e = np.ascontiguousarray(
        np.concatenate([Wh2[:, I0:I1], Wh2[:, I2:]], axis=1) * SW
    ).astype(bf)
    BW1 = np.ascontiguousarray(
        (bh2[:I0] * SW).reshape(2, 128, M).transpose(1, 0, 2)
    ).astype(bf)
    BW2 = np.ascontiguousarray(
        (bh2[I1:I2] * SW).reshape(2, 128, D).transpose(1, 0, 2)
    ).astype(bf)
    bb1 = (bh2[I0:I1] * SW)[None].astype(bf)
    bb2 = (bh2[I2:] * SW)[None].astype(bf)
    return {"Wh2s8": Wh2s8, "Wh2e": Wh2e, "BW1": BW1, "BW2": BW2,
            "bb1": bb1, "bb2": bb2}


def _in_maps(inputs):
    full = {k: np.ascontiguousarray(np.asarray(v, dtype=np.float32))
            for k, v in inputs.items()}
    staged = _stage(full.pop("Wh2"), full.pop("bh2"))
    full.update(staged)
    maps = []
    for i in range(NCORES):
        m = dict(full)
        m["x"] = full["x"][i * BS:(i + 1) * BS]
        m["context"] = full["context"][i * BS:(i + 1) * BS]
        maps.append(m)
    return maps


def _get_nc():
    global _CACHED_NC
    if _CACHED_NC is None:
        _CACHED_NC = build_nc()
    return _CACHED_NC


def run_spmd(inputs, trace=False):
    from concourse.bass_utils import run_bass_kernel_spmd

    nc = _get_nc()
    res = run_bass_kernel_spmd(nc, _in_maps(inputs), list(range(NCORES)), trace=trace)
    out = np.concatenate([res.results[i]["out"] for i in range(NCORES)], axis=0)
    return out, res


def kernel(**inputs) -> np.ndarray:
    out, _ = run_spmd(inputs, trace=False)
    return out
